# revision 27
# baseline (speedup 1.0000x reference)
"""Multi-head attention (B=2, S=2048, D=1024, H=16) on 8 TRN2 NeuronCores.

Sharding: tensor-parallel over heads. Core c computes heads {2c, 2c+1}:
  - q/k/v projections restricted to its 128 output dims (full q/k/v
    streamed per core, rank-128 weight slices),
  - causal flash attention for its 2 heads (both batches), scores kept in
    [k_token, q_token] layout so no transposes are needed,
  - partial output projection with its 128-column slice of Wo.
Host sums the 8 partial outputs.

All matmuls run as float32r (TRN2's full-rate fp32 mode; ~1.5e-4 rel err
vs float64 at K=1024 — measured identical to the PE's plain fp32 mode,
at 1 cycle/row instead of 4). Softmax skips the max-subtraction (scores
are O(+-2) for sane inputs) and gets denominators for free from a
ones-augmented V (M=65 matmul: rows 0-63 = sum(P*V), row 64 = sum(P)).
Causal masking zeroes the exp tiles on diagonal-crossing k-tiles with a
single GpSimd affine_select each; fully-masked tiles are skipped. The
mask handling is derived from the actual mask input on the host, so any
mask pattern (causal, full, block-sparse, non-affine) works.

The emission schedule is a single fused software pipeline per
(batch, 512-token q-block): scores->exp->PV beats with the AV matmul
lagging one beat behind the exp, and the next chunk's projection runs,
the previous block's normalize + output projection, and the V-transpose
interleaved into the beats so the PE instruction stream stays dense
(keeps the PE_HAM clock gate at full speed).
"""

import sys
import types

import numpy as np

# If tracing is requested (BASS_TRACE) but the image's antenv lacks the
# axon_hooks module, register a ctypes-based NTFF hook so the trace path
# in run_bass_kernel_spmd does not crash.
try:
    import antenv.axon_hooks  # noqa: F401
except ImportError:
    try:
        from trn_agent_boot.trn_boot import _ntff_profile_via_ctypes
        _m = types.ModuleType("antenv.axon_hooks")
        _hook = _ntff_profile_via_ctypes("/opt/axon/libaxon_pjrt.so")
        _m.get_axon_ntff_profile_hook = lambda: _hook
        _m.set_axon_ntff_profile_hook = lambda h: None
        sys.modules["antenv.axon_hooks"] = _m
    except Exception:
        pass

import concourse.mybir as mybir
from concourse import bacc
from concourse.tile import TileContext
from concourse.bass_utils import run_bass_kernel_spmd
from concourse.masks import make_identity

B, S, D, H, HD = 2, 2048, 1024, 16, 64
N_CORES = 8
DLOC = D // N_CORES          # 128 dims (2 heads) per core
SCALE = 1.0 / np.sqrt(HD)
TBLK = 512                   # q-token block (ST free dim)
JBLK = 128                   # k-token tile (ST partition dim)
NTB = S // TBLK              # 4 t-blocks per batch
NJT = S // JBLK              # 16 j-tiles per batch
ET = D // 128                # 8 contraction e-tiles
NEG = -1.0e30

F32 = mybir.dt.float32
F32R = mybir.dt.float32r

_compiled = {}


def _classify(mask2d):
    """Per (t-block, j-tile): None (fully valid), skipped (fully masked),
    ("affine", r) when the valid set is {(jj,tt): tt >= jj + r},
    else ("bias", idx) into a dedup'd list of [128 j, 512 t] bias tiles."""
    pats = {}
    pat_list = []
    table = []
    jj = np.arange(JBLK)[:, None]
    tt = np.arange(TBLK)[None, :]
    for tb in range(NTB):
        t0 = tb * TBLK
        row = []
        for j0 in range(0, S, JBLK):
            blk = mask2d[t0:t0 + TBLK, j0:j0 + JBLK]  # [t, j]
            if blk.all():
                row.append((j0, None))
                continue
            if not blk.any():
                continue
            valid = blk.T != 0  # [jj, tt]
            # affine candidate: r from the first row's first valid column
            first = np.argmax(valid, axis=1)  # per jj: first valid tt (if any)
            r = int(first[0])
            if np.array_equal(valid, tt >= jj + r):
                row.append((j0, ("affine", r)))
            else:
                bias = np.where(valid, np.float32(0.0),
                                np.float32(NEG)).astype(np.float32)
                key = bias.tobytes()
                if key not in pats:
                    pats[key] = len(pat_list)
                    pat_list.append(bias)
                row.append((j0, ("bias", pats[key])))
        table.append(tuple(row))
    return tuple(table), pat_list


def _build(table, n_pat):
    nc = bacc.Bacc(None, num_devices=N_CORES)
    qT_ext = nc.declare_dram_parameter("qT", [D, B * S], F32R, isOutput=False)
    kT_ext = nc.declare_dram_parameter("kT", [D, B * S], F32R, isOutput=False)
    vT_ext = nc.declare_dram_parameter("vT", [D, B * S], F32R, isOutput=False)
    wq_ext = nc.declare_dram_parameter("wq", [D, DLOC], F32R, isOutput=False)
    wk_ext = nc.declare_dram_parameter("wk", [D, DLOC], F32R, isOutput=False)
    wv_ext = nc.declare_dram_parameter("wv", [D, DLOC], F32R, isOutput=False)
    wo_ext = nc.declare_dram_parameter("wo", [DLOC, D], F32R, isOutput=False)
    if n_pat:
        bias_ext = nc.declare_dram_parameter("bias", [JBLK, n_pat * TBLK], F32,
                                             isOutput=False)
    pout_ext = nc.declare_dram_parameter("pout", [D, B * S], F32, isOutput=True)

    with TileContext(nc) as tc:
        with (
            tc.tile_pool(name="const", bufs=1) as const,
            tc.tile_pool(name="big", bufs=1) as big,
            tc.tile_pool(name="inp", bufs=2) as inp,
            tc.tile_pool(name="work", bufs=3) as work,
            tc.tile_pool(name="ps", bufs=1, space="PSUM") as ps,
        ):
            # ---------- constants ----------
            wq_sb = const.tile([128, ET * DLOC], F32R, tag="wq")
            wk_sb = const.tile([128, ET * DLOC], F32R, tag="wk")
            wv_sb = const.tile([128, ET * DLOC], F32R, tag="wv")
            for w_sb, w_ext in ((wq_sb, wq_ext), (wk_sb, wk_ext), (wv_sb, wv_ext)):
                nc.sync.dma_start(
                    out=w_sb[:].rearrange("p (a d) -> p a d", a=ET),
                    in_=w_ext.rearrange("(a p) d -> p a d", p=128))
            wo_sb = const.tile([DLOC, D], F32R, tag="wo")
            nc.sync.dma_start(out=wo_sb[:], in_=wo_ext[:, :])
            if n_pat:
                bias_sb = const.tile([JBLK, n_pat * TBLK], F32, tag="bias")
                nc.sync.dma_start(out=bias_sb[:], in_=bias_ext[:, :])
            ident = const.tile([128, 128], F32, tag="ident")
            make_identity(nc, ident[:])
            ones32 = const.tile([128, NJT], F32, tag="ones32")
            nc.vector.memset(ones32[:], 1.0)

            # ---------- persistent per-core activations ----------
            qhT_sb = big.tile([128, B * S], F32R, tag="qhT")
            khT_sb = big.tile([128, B * S], F32R, tag="khT")
            vaug_sb = big.tile([128, B * 2 * NJT * 65], F32R, tag="vaug")
            attn2_sb = big.tile([128, B * S], F32R, tag="attn2")

            def vaug_sl(b, h, j0, c0, c1):
                base = ((b * 2 + h) * NJT + j0 // JBLK) * 65
                return vaug_sb[:, base + c0:base + c1]

            for b in range(B):
                for h in range(2):
                    base = (b * 2 + h) * NJT * 65 + 64
                    dst = vaug_sb[:, base:base + 65 * (NJT - 1) + 1:65]
                    nc.vector.tensor_copy(dst, ones32[:])

            av_tiles = {}

            # ---------- building blocks ----------
            def chunk_dmas_one(b, tch, nm, ext):
                col0 = b * S + tch * TBLK
                grp = []
                for e in range(ET):
                    t_in = inp.tile([128, TBLK], F32R, tag="xin", bufs=24,
                                    name=f"in_{nm}_{b}_{tch}_{e}")
                    nc.sync.dma_start(
                        out=t_in[:],
                        in_=ext[e * 128:(e + 1) * 128, col0:col0 + TBLK])
                    grp.append(t_in)
                return {nm: grp}

            def chunk_dmas(b, tch):
                """Queue the 24 input-tile DMAs for chunk (b, tch)."""
                tiles = {}
                for nm, ext in (("q", qT_ext), ("k", kT_ext), ("v", vT_ext)):
                    tiles.update(chunk_dmas_one(b, tch, nm, ext))
                return tiles

            def proj_half(b, tch, nm, tiles, half, state):
                """Half of a projection accumulation run (4 matmuls); the
                second half runs the epilogue."""
                col0 = b * S + tch * TBLK
                w_sb = {"q": wq_sb, "k": wk_sb, "v": wv_sb}[nm]
                if half == 0:
                    pp = ps.tile([128, TBLK], F32, tag="pc", bufs=2,
                                 name=f"pp_{nm}_{b}_{tch}")
                    state[nm] = pp
                else:
                    pp = state[nm]
                for e in range(half * 4, half * 4 + 4):
                    nc.tensor.matmul(pp[:], lhsT=w_sb[:, e * DLOC:(e + 1) * DLOC],
                                     rhs=tiles[e][:],
                                     start=(e == 0), stop=(e == ET - 1))
                if half == 0:
                    return
                if nm == "q":
                    nc.vector.tensor_copy(qhT_sb[:, col0:col0 + TBLK], pp[:])
                elif nm == "k":
                    nc.vector.tensor_copy(khT_sb[:, col0:col0 + TBLK], pp[:])
                else:
                    vt = work.tile([128, TBLK], F32, tag="vt", bufs=2)
                    nc.vector.tensor_copy(vt[:], pp[:])
                    for kb in range(TBLK // 128):
                        tr = ps.tile([128, 128], F32, tag="pc", bufs=2,
                                     name=f"tr_{b}_{tch}_{kb}")
                        nc.tensor.transpose(tr[:], vt[:, kb * 128:(kb + 1) * 128],
                                            ident[:])
                        j0 = tch * TBLK + kb * 128
                        nc.vector.tensor_copy(vaug_sl(b, 0, j0, 0, 64), tr[:, 0:64])
                        nc.vector.tensor_copy(vaug_sl(b, 1, j0, 0, 64), tr[:, 64:128])

            def fin_normalize(b, tb):
                """Divide by the ones-row denominator into attn2_sb; head B
                lands on partitions 64-127 via a small SBUF shift DMA."""
                cc = b * S + tb * TBLK
                av = av_tiles[(b, tb)]
                for h in range(2):
                    den = work.tile([65, TBLK], F32, tag="den", bufs=2)
                    nc.vector.tensor_copy(den[64:65, :], av[h][64:65, :])
                    sh = work.tile([1, TBLK], F32, tag="sh", bufs=2)
                    nc.gpsimd.dma_start(out=sh[:], in_=den[64:65, :])
                    bc = work.tile([64, TBLK], F32, tag="bc", bufs=2)
                    nc.gpsimd.partition_broadcast(bc[:], sh[:])
                    bcr = work.tile([64, TBLK], F32, tag="bcr", bufs=2)
                    nc.vector.reciprocal_approx_fast(bcr[:], bc[:])
                    if h == 0:
                        nc.vector.tensor_mul(attn2_sb[0:64, cc:cc + TBLK],
                                             av[h][0:64, :], bcr[:])
                    else:
                        tmpb = work.tile([64, TBLK], F32R, tag="tmpb", bufs=2)
                        nc.vector.tensor_mul(tmpb[:], av[h][0:64, :], bcr[:])
                        nc.gpsimd.dma_start(out=attn2_sb[64:128, cc:cc + TBLK],
                                            in_=tmpb[:])

            def po_group(b, tb, e):
                """Partial out-projection for e-tile e of block (b, tb)."""
                cc = b * S + tb * TBLK
                po = ps.tile([128, TBLK], F32, tag="pc", bufs=2,
                             name=f"po_{b}_{tb}_{e}")
                nc.tensor.matmul(po[:], lhsT=wo_sb[:, e * 128:(e + 1) * 128],
                                 rhs=attn2_sb[:, cc:cc + TBLK],
                                 start=True, stop=True)
                pos = work.tile([128, TBLK], F32, tag="pos", bufs=3)
                nc.vector.tensor_copy(pos[:], po[:])
                nc.gpsimd.dma_start(out=pout_ext[e * 128:(e + 1) * 128, cc:cc + TBLK],
                                    in_=pos[:])

            # ---------- fused pipeline ----------
            def block(b, tb, prev, nxt):
                """Attention for (b, tb) with prev's finish and nxt's
                projection interleaved into the beats."""
                cc = b * S + tb * TBLK
                row = table[tb]
                n_j = len(row)
                if prev is not None:
                    fin_normalize(*prev)
                fillers = []
                if nxt is not None:
                    tiles = chunk_dmas(*nxt)
                    pstate = {}
                    for nm in ("q", "k", "v"):
                        for half in range(2):
                            fillers.append(
                                lambda nm=nm, half=half:
                                proj_half(nxt[0], nxt[1], nm, tiles[nm], half, pstate))
                if prev is not None:
                    for e in range(ET):
                        fillers.append(lambda pb=prev[0], pt=prev[1], pe=e:
                                       po_group(pb, pt, pe))
                av = [ps.tile([65, TBLK], F32, tag="p1a", name=f"avA_{b}_{tb}"),
                      ps.tile([65, TBLK], F32, tag="p1b", name=f"avB_{b}_{tb}")]
                av_tiles[(b, tb)] = av

                pend = []
                nfill = len(fillers)
                fi = 0
                for ji, (j0, m) in enumerate(row):
                    jc = b * S + j0
                    # leading fully-masked columns of an affine tile: skip
                    r = m[1] if (m is not None and m[0] == "affine") else 0
                    r = max(0, min(r, TBLK - 128))  # keep at least 128 cols
                    stA = ps.tile([128, TBLK], F32, tag="p2a", bufs=2)
                    stB = ps.tile([128, TBLK], F32, tag="p2b", bufs=2)
                    nc.tensor.matmul(stA[:, r:], lhsT=khT_sb[0:64, jc:jc + JBLK],
                                     rhs=qhT_sb[0:64, cc + r:cc + TBLK],
                                     start=True, stop=True)
                    nc.tensor.matmul(stB[:, r:], lhsT=khT_sb[64:128, jc:jc + JBLK],
                                     rhs=qhT_sb[64:128, cc + r:cc + TBLK],
                                     start=True, stop=True)
                    exps = []
                    for h, sth in enumerate((stA, stB)):
                        if m is not None and m[0] == "bias":
                            nc.vector.tensor_add(
                                sth[:], sth[:],
                                bias_sb[:, m[1] * TBLK:(m[1] + 1) * TBLK])
                        ex = work.tile([128, TBLK], F32R, tag=f"exp{h}", bufs=4)
                        nc.scalar.activation(ex[:, r:], sth[:, r:],
                                             mybir.ActivationFunctionType.Exp,
                                             scale=float(SCALE))
                        if m is not None and m[0] == "affine":
                            # keep element iff (tt-r) >= jj + (m[1]-r), else 0
                            nc.gpsimd.affine_select(
                                out=ex[:, r:], in_=ex[:, r:],
                                pattern=[[1, TBLK - r]],
                                compare_op=mybir.AluOpType.is_ge,
                                fill=0.0, base=-(m[1] - r), channel_multiplier=-1)
                        exps.append(ex)
                    pend.append((exps, j0, ji, r))
                    if len(pend) > 2:
                        pexps, pj0, pji, pr = pend.pop(0)
                        for h in range(2):
                            nc.tensor.matmul(av[h][:, pr:],
                                             lhsT=vaug_sl(b, h, pj0, 0, 65),
                                             rhs=pexps[h][:, pr:],
                                             start=(pji == 0), stop=False)
                    # interleave filler work to keep the PE stream dense
                    want = (ji + 1) * nfill // (n_j + 2)
                    while fi < want:
                        fillers[fi]()
                        fi += 1
                while pend:
                    pexps, pj0, pji, pr = pend.pop(0)
                    for h in range(2):
                        nc.tensor.matmul(av[h][:, pr:],
                                         lhsT=vaug_sl(b, h, pj0, 0, 65),
                                         rhs=pexps[h][:, pr:],
                                         start=(pji == 0), stop=(pji == n_j - 1))
                while fi < nfill:
                    fillers[fi]()
                    fi += 1

            # prologue: project chunk (0, 0)
            tiles0 = chunk_dmas(0, 0)
            pstate0 = {}
            for nm in ("q", "k", "v"):
                for half in range(2):
                    proj_half(0, 0, nm, tiles0[nm], half, pstate0)
            # fused blocks
            blocks = [(b, tb) for b in range(B) for tb in range(NTB)]
            for i, (b, tb) in enumerate(blocks):
                prev = blocks[i - 1] if i > 0 else None
                nxt = blocks[i + 1] if i + 1 < len(blocks) else None
                block(b, tb, prev, nxt)
            # epilogue: finish the last block
            last = blocks[-1]
            fin_normalize(*last)
            for e in range(ET):
                po_group(last[0], last[1], e)

    nc.finalize()
    return nc


def kernel(q, k, v, mask, Wq, Wk, Wv, Wo):
    q = np.asarray(q, dtype=np.float32)
    k = np.asarray(k, dtype=np.float32)
    v = np.asarray(v, dtype=np.float32)
    mask2d = np.asarray(mask).reshape(S, S)
    Wq = np.asarray(Wq, dtype=np.float32)
    Wk = np.asarray(Wk, dtype=np.float32)
    Wv = np.asarray(Wv, dtype=np.float32)
    Wo = np.asarray(Wo, dtype=np.float32)

    table, pat_list = _classify(mask2d)
    n_pat = len(pat_list)
    key = (table, n_pat)
    if key not in _compiled:
        _compiled[key] = _build(table, n_pat)
    nc = _compiled[key]

    qT = np.ascontiguousarray(q.reshape(B * S, D).T)
    kT = np.ascontiguousarray(k.reshape(B * S, D).T)
    vT = np.ascontiguousarray(v.reshape(B * S, D).T)
    if n_pat:
        bias_cat = np.concatenate(pat_list, axis=1)

    in_maps = []
    for c in range(N_CORES):
        sl = slice(c * DLOC, (c + 1) * DLOC)
        m = {
            "qT": qT, "kT": kT, "vT": vT,
            "wq": np.ascontiguousarray(Wq[sl, :].T),
            "wk": np.ascontiguousarray(Wk[sl, :].T),
            "wv": np.ascontiguousarray(Wv[sl, :].T),
            "wo": np.ascontiguousarray(Wo[:, sl].T),
        }
        if n_pat:
            m["bias"] = bias_cat
        in_maps.append(m)

    res = run_bass_kernel_spmd(nc, in_maps, list(range(N_CORES)))
    global last_run
    last_run = res
    out_T = np.zeros((D, B * S), dtype=np.float32)
    for c in range(N_CORES):
        out_T += res.results[c]["pout"]
    return np.ascontiguousarray(out_T.T).reshape(B, S, D)


last_run = None


# revision 28
# speedup vs baseline: 1.0042x; 1.0042x over previous
"""Multi-head attention (B=2, S=2048, D=1024, H=16) on 8 TRN2 NeuronCores.

Sharding: tensor-parallel over heads. Core c computes heads {2c, 2c+1}:
  - q/k/v projections restricted to its 128 output dims (full q/k/v
    streamed per core, rank-128 weight slices),
  - causal flash attention for its 2 heads (both batches), scores kept in
    [k_token, q_token] layout so no transposes are needed,
  - partial output projection with its 128-column slice of Wo.
Host sums the 8 partial outputs.

All matmuls run as float32r (TRN2's full-rate fp32 mode; ~1.5e-4 rel err
vs float64 at K=1024 — measured identical to the PE's plain fp32 mode,
at 1 cycle/row instead of 4). Softmax skips the max-subtraction (scores
are O(+-2) for sane inputs) and gets denominators for free from a
ones-augmented V (M=65 matmul: rows 0-63 = sum(P*V), row 64 = sum(P)).
Causal masking zeroes the exp tiles on diagonal-crossing k-tiles with a
single GpSimd affine_select each; fully-masked tiles are skipped. The
mask handling is derived from the actual mask input on the host, so any
mask pattern (causal, full, block-sparse, non-affine) works.

The emission schedule is a single fused software pipeline per
(batch, 512-token q-block): scores->exp->PV beats with the AV matmul
lagging one beat behind the exp, and the next chunk's projection runs,
the previous block's normalize + output projection, and the V-transpose
interleaved into the beats so the PE instruction stream stays dense
(keeps the PE_HAM clock gate at full speed).
"""

import sys
import types

import numpy as np

# If tracing is requested (BASS_TRACE) but the image's antenv lacks the
# axon_hooks module, register a ctypes-based NTFF hook so the trace path
# in run_bass_kernel_spmd does not crash.
try:
    import antenv.axon_hooks  # noqa: F401
except ImportError:
    try:
        from trn_agent_boot.trn_boot import _ntff_profile_via_ctypes
        _m = types.ModuleType("antenv.axon_hooks")
        _hook = _ntff_profile_via_ctypes("/opt/axon/libaxon_pjrt.so")
        _m.get_axon_ntff_profile_hook = lambda: _hook
        _m.set_axon_ntff_profile_hook = lambda h: None
        sys.modules["antenv.axon_hooks"] = _m
    except Exception:
        pass

import concourse.mybir as mybir
from concourse import bacc
from concourse.tile import TileContext
from concourse.bass_utils import run_bass_kernel_spmd
from concourse.masks import make_identity

B, S, D, H, HD = 2, 2048, 1024, 16, 64
N_CORES = 8
DLOC = D // N_CORES          # 128 dims (2 heads) per core
SCALE = 1.0 / np.sqrt(HD)
TBLK = 512                   # q-token block (ST free dim)
JBLK = 128                   # k-token tile (ST partition dim)
NTB = S // TBLK              # 4 t-blocks per batch
NJT = S // JBLK              # 16 j-tiles per batch
ET = D // 128                # 8 contraction e-tiles
NEG = -1.0e30

F32 = mybir.dt.float32
F32R = mybir.dt.float32r

_compiled = {}


def _classify(mask2d):
    """Per (t-block, j-tile): None (fully valid), skipped (fully masked),
    ("affine", r) when the valid set is {(jj,tt): tt >= jj + r},
    else ("bias", idx) into a dedup'd list of [128 j, 512 t] bias tiles."""
    pats = {}
    pat_list = []
    table = []
    jj = np.arange(JBLK)[:, None]
    tt = np.arange(TBLK)[None, :]
    for tb in range(NTB):
        t0 = tb * TBLK
        row = []
        for j0 in range(0, S, JBLK):
            blk = mask2d[t0:t0 + TBLK, j0:j0 + JBLK]  # [t, j]
            if blk.all():
                row.append((j0, None))
                continue
            if not blk.any():
                continue
            valid = blk.T != 0  # [jj, tt]
            # affine candidate: r from the first row's first valid column
            first = np.argmax(valid, axis=1)  # per jj: first valid tt (if any)
            r = int(first[0])
            if np.array_equal(valid, tt >= jj + r):
                row.append((j0, ("affine", r)))
            else:
                bias = np.where(valid, np.float32(0.0),
                                np.float32(NEG)).astype(np.float32)
                key = bias.tobytes()
                if key not in pats:
                    pats[key] = len(pat_list)
                    pat_list.append(bias)
                row.append((j0, ("bias", pats[key])))
        table.append(tuple(row))
    return tuple(table), pat_list


def _build(table, n_pat):
    nc = bacc.Bacc(None, num_devices=N_CORES)
    qT_ext = nc.declare_dram_parameter("qT", [D, B * S], F32R, isOutput=False)
    kT_ext = nc.declare_dram_parameter("kT", [D, B * S], F32R, isOutput=False)
    vT_ext = nc.declare_dram_parameter("vT", [D, B * S], F32R, isOutput=False)
    wq_ext = nc.declare_dram_parameter("wq", [D, DLOC], F32R, isOutput=False)
    wk_ext = nc.declare_dram_parameter("wk", [D, DLOC], F32R, isOutput=False)
    wv_ext = nc.declare_dram_parameter("wv", [D, DLOC], F32R, isOutput=False)
    wo_ext = nc.declare_dram_parameter("wo", [DLOC, D], F32R, isOutput=False)
    if n_pat:
        bias_ext = nc.declare_dram_parameter("bias", [JBLK, n_pat * TBLK], F32,
                                             isOutput=False)
    pout_ext = nc.declare_dram_parameter("pout", [D, B * S], F32, isOutput=True)

    with TileContext(nc) as tc:
        with (
            tc.tile_pool(name="const", bufs=1) as const,
            tc.tile_pool(name="big", bufs=1) as big,
            tc.tile_pool(name="inp", bufs=2) as inp,
            tc.tile_pool(name="work", bufs=3) as work,
            tc.tile_pool(name="ps", bufs=1, space="PSUM") as ps,
        ):
            # ---------- constants ----------
            wq_sb = const.tile([128, ET * DLOC], F32R, tag="wq")
            wk_sb = const.tile([128, ET * DLOC], F32R, tag="wk")
            wv_sb = const.tile([128, ET * DLOC], F32R, tag="wv")
            for w_sb, w_ext in ((wq_sb, wq_ext), (wk_sb, wk_ext), (wv_sb, wv_ext)):
                nc.sync.dma_start(
                    out=w_sb[:].rearrange("p (a d) -> p a d", a=ET),
                    in_=w_ext.rearrange("(a p) d -> p a d", p=128))
            wo_sb = const.tile([DLOC, D], F32R, tag="wo")
            nc.sync.dma_start(out=wo_sb[:], in_=wo_ext[:, :])
            if n_pat:
                bias_sb = const.tile([JBLK, n_pat * TBLK], F32, tag="bias")
                nc.sync.dma_start(out=bias_sb[:], in_=bias_ext[:, :])
            ident = const.tile([128, 128], F32, tag="ident")
            make_identity(nc, ident[:])
            ones32 = const.tile([128, NJT], F32, tag="ones32")
            nc.vector.memset(ones32[:], 1.0)

            # ---------- persistent per-core activations ----------
            qhT_sb = big.tile([128, B * S], F32R, tag="qhT")
            khT_sb = big.tile([128, B * S], F32R, tag="khT")
            vaug_sb = big.tile([128, B * 2 * NJT * 65], F32R, tag="vaug")
            attn2_sb = big.tile([128, B * S], F32R, tag="attn2")

            def vaug_sl(b, h, j0, c0, c1):
                base = ((b * 2 + h) * NJT + j0 // JBLK) * 65
                return vaug_sb[:, base + c0:base + c1]

            for b in range(B):
                for h in range(2):
                    base = (b * 2 + h) * NJT * 65 + 64
                    dst = vaug_sb[:, base:base + 65 * (NJT - 1) + 1:65]
                    nc.vector.tensor_copy(dst, ones32[:])

            av_tiles = {}

            # ---------- building blocks ----------
            def chunk_dmas_one(b, tch, nm, ext):
                col0 = b * S + tch * TBLK
                grp = []
                for e in range(ET):
                    t_in = inp.tile([128, TBLK], F32R, tag="xin", bufs=24,
                                    name=f"in_{nm}_{b}_{tch}_{e}")
                    nc.sync.dma_start(
                        out=t_in[:],
                        in_=ext[e * 128:(e + 1) * 128, col0:col0 + TBLK])
                    grp.append(t_in)
                return {nm: grp}

            def chunk_dmas(b, tch):
                """Queue the 24 input-tile DMAs for chunk (b, tch)."""
                tiles = {}
                for nm, ext in (("q", qT_ext), ("k", kT_ext), ("v", vT_ext)):
                    tiles.update(chunk_dmas_one(b, tch, nm, ext))
                return tiles

            def proj_half(b, tch, nm, tiles, half, state):
                """Half of a projection accumulation run (4 matmuls); the
                second half runs the epilogue."""
                col0 = b * S + tch * TBLK
                w_sb = {"q": wq_sb, "k": wk_sb, "v": wv_sb}[nm]
                if half == 0:
                    pp = ps.tile([128, TBLK], F32, tag="pc", bufs=2,
                                 name=f"pp_{nm}_{b}_{tch}")
                    state[nm] = pp
                else:
                    pp = state[nm]
                for e in range(half * 4, half * 4 + 4):
                    nc.tensor.matmul(pp[:], lhsT=w_sb[:, e * DLOC:(e + 1) * DLOC],
                                     rhs=tiles[e][:],
                                     start=(e == 0), stop=(e == ET - 1))
                if half == 0:
                    return
                if nm == "q":
                    nc.vector.tensor_copy(qhT_sb[:, col0:col0 + TBLK], pp[:])
                elif nm == "k":
                    nc.vector.tensor_copy(khT_sb[:, col0:col0 + TBLK], pp[:])
                else:
                    vt = work.tile([128, TBLK], F32, tag="vt", bufs=2)
                    nc.vector.tensor_copy(vt[:], pp[:])
                    for kb in range(TBLK // 128):
                        tr = ps.tile([128, 128], F32, tag="pc", bufs=2,
                                     name=f"tr_{b}_{tch}_{kb}")
                        nc.tensor.transpose(tr[:], vt[:, kb * 128:(kb + 1) * 128],
                                            ident[:])
                        j0 = tch * TBLK + kb * 128
                        nc.vector.tensor_copy(vaug_sl(b, 0, j0, 0, 64), tr[:, 0:64])
                        nc.vector.tensor_copy(vaug_sl(b, 1, j0, 0, 64), tr[:, 64:128])

            def fin_normalize(b, tb):
                """Divide by the ones-row denominator into attn2_sb; head B
                lands on partitions 64-127 via a small SBUF shift DMA."""
                cc = b * S + tb * TBLK
                av = av_tiles[(b, tb)]
                for h in range(2):
                    den = work.tile([65, TBLK], F32, tag="den", bufs=2)
                    nc.vector.tensor_copy(den[64:65, :], av[h][64:65, :])
                    sh = work.tile([1, TBLK], F32, tag="sh", bufs=2)
                    nc.sync.dma_start(out=sh[:], in_=den[64:65, :])
                    bc = work.tile([64, TBLK], F32, tag="bc", bufs=2)
                    nc.gpsimd.partition_broadcast(bc[:], sh[:])
                    bcr = work.tile([64, TBLK], F32, tag="bcr", bufs=2)
                    nc.vector.reciprocal_approx_fast(bcr[:], bc[:])
                    if h == 0:
                        nc.vector.tensor_mul(attn2_sb[0:64, cc:cc + TBLK],
                                             av[h][0:64, :], bcr[:])
                    else:
                        tmpb = work.tile([64, TBLK], F32R, tag="tmpb", bufs=2)
                        nc.vector.tensor_mul(tmpb[:], av[h][0:64, :], bcr[:])
                        nc.sync.dma_start(out=attn2_sb[64:128, cc:cc + TBLK],
                                          in_=tmpb[:])

            def po_group(b, tb, e):
                """Partial out-projection for e-tile e of block (b, tb)."""
                cc = b * S + tb * TBLK
                po = ps.tile([128, TBLK], F32, tag="pc", bufs=2,
                             name=f"po_{b}_{tb}_{e}")
                nc.tensor.matmul(po[:], lhsT=wo_sb[:, e * 128:(e + 1) * 128],
                                 rhs=attn2_sb[:, cc:cc + TBLK],
                                 start=True, stop=True)
                pos = work.tile([128, TBLK], F32, tag="pos", bufs=3)
                nc.vector.tensor_copy(pos[:], po[:])
                nc.gpsimd.dma_start(out=pout_ext[e * 128:(e + 1) * 128, cc:cc + TBLK],
                                    in_=pos[:])

            # ---------- fused pipeline ----------
            def block(b, tb, prev, nxt):
                """Attention for (b, tb) with prev's finish and nxt's
                projection interleaved into the beats."""
                cc = b * S + tb * TBLK
                row = table[tb]
                n_j = len(row)
                if prev is not None:
                    fin_normalize(*prev)
                fillers = []
                if nxt is not None:
                    tiles = chunk_dmas(*nxt)
                    pstate = {}
                    for nm in ("q", "k", "v"):
                        for half in range(2):
                            fillers.append(
                                lambda nm=nm, half=half:
                                proj_half(nxt[0], nxt[1], nm, tiles[nm], half, pstate))
                if prev is not None:
                    for e in range(ET):
                        fillers.append(lambda pb=prev[0], pt=prev[1], pe=e:
                                       po_group(pb, pt, pe))
                av = [ps.tile([65, TBLK], F32, tag="p1a", name=f"avA_{b}_{tb}"),
                      ps.tile([65, TBLK], F32, tag="p1b", name=f"avB_{b}_{tb}")]
                av_tiles[(b, tb)] = av

                pend = []
                nfill = len(fillers)
                fi = 0
                for ji, (j0, m) in enumerate(row):
                    jc = b * S + j0
                    # leading fully-masked columns of an affine tile: skip
                    r = m[1] if (m is not None and m[0] == "affine") else 0
                    r = max(0, min(r, TBLK - 128))  # keep at least 128 cols
                    stA = ps.tile([128, TBLK], F32, tag="p2a", bufs=2)
                    stB = ps.tile([128, TBLK], F32, tag="p2b", bufs=2)
                    nc.tensor.matmul(stA[:, r:], lhsT=khT_sb[0:64, jc:jc + JBLK],
                                     rhs=qhT_sb[0:64, cc + r:cc + TBLK],
                                     start=True, stop=True)
                    nc.tensor.matmul(stB[:, r:], lhsT=khT_sb[64:128, jc:jc + JBLK],
                                     rhs=qhT_sb[64:128, cc + r:cc + TBLK],
                                     start=True, stop=True)
                    exps = []
                    for h, sth in enumerate((stA, stB)):
                        if m is not None and m[0] == "bias":
                            nc.vector.tensor_add(
                                sth[:], sth[:],
                                bias_sb[:, m[1] * TBLK:(m[1] + 1) * TBLK])
                        ex = work.tile([128, TBLK], F32R, tag=f"exp{h}", bufs=4)
                        nc.scalar.activation(ex[:, r:], sth[:, r:],
                                             mybir.ActivationFunctionType.Exp,
                                             scale=float(SCALE))
                        if m is not None and m[0] == "affine":
                            # keep element iff (tt-r) >= jj + (m[1]-r), else 0
                            nc.gpsimd.affine_select(
                                out=ex[:, r:], in_=ex[:, r:],
                                pattern=[[1, TBLK - r]],
                                compare_op=mybir.AluOpType.is_ge,
                                fill=0.0, base=-(m[1] - r), channel_multiplier=-1)
                        exps.append(ex)
                    pend.append((exps, j0, ji, r))
                    if len(pend) > 2:
                        pexps, pj0, pji, pr = pend.pop(0)
                        for h in range(2):
                            nc.tensor.matmul(av[h][:, pr:],
                                             lhsT=vaug_sl(b, h, pj0, 0, 65),
                                             rhs=pexps[h][:, pr:],
                                             start=(pji == 0), stop=False)
                    # interleave filler work to keep the PE stream dense
                    want = (ji + 1) * nfill // (n_j + 2)
                    while fi < want:
                        fillers[fi]()
                        fi += 1
                while pend:
                    pexps, pj0, pji, pr = pend.pop(0)
                    for h in range(2):
                        nc.tensor.matmul(av[h][:, pr:],
                                         lhsT=vaug_sl(b, h, pj0, 0, 65),
                                         rhs=pexps[h][:, pr:],
                                         start=(pji == 0), stop=(pji == n_j - 1))
                while fi < nfill:
                    fillers[fi]()
                    fi += 1

            # prologue: project chunk (0, 0)
            tiles0 = chunk_dmas(0, 0)
            pstate0 = {}
            for nm in ("q", "k", "v"):
                for half in range(2):
                    proj_half(0, 0, nm, tiles0[nm], half, pstate0)
            # fused blocks
            blocks = [(b, tb) for b in range(B) for tb in range(NTB)]
            for i, (b, tb) in enumerate(blocks):
                prev = blocks[i - 1] if i > 0 else None
                nxt = blocks[i + 1] if i + 1 < len(blocks) else None
                block(b, tb, prev, nxt)
            # epilogue: finish the last block
            last = blocks[-1]
            fin_normalize(*last)
            for e in range(ET):
                po_group(last[0], last[1], e)

    nc.finalize()
    return nc


def kernel(q, k, v, mask, Wq, Wk, Wv, Wo):
    q = np.asarray(q, dtype=np.float32)
    k = np.asarray(k, dtype=np.float32)
    v = np.asarray(v, dtype=np.float32)
    mask2d = np.asarray(mask).reshape(S, S)
    Wq = np.asarray(Wq, dtype=np.float32)
    Wk = np.asarray(Wk, dtype=np.float32)
    Wv = np.asarray(Wv, dtype=np.float32)
    Wo = np.asarray(Wo, dtype=np.float32)

    table, pat_list = _classify(mask2d)
    n_pat = len(pat_list)
    key = (table, n_pat)
    if key not in _compiled:
        _compiled[key] = _build(table, n_pat)
    nc = _compiled[key]

    qT = np.ascontiguousarray(q.reshape(B * S, D).T)
    kT = np.ascontiguousarray(k.reshape(B * S, D).T)
    vT = np.ascontiguousarray(v.reshape(B * S, D).T)
    if n_pat:
        bias_cat = np.concatenate(pat_list, axis=1)

    in_maps = []
    for c in range(N_CORES):
        sl = slice(c * DLOC, (c + 1) * DLOC)
        m = {
            "qT": qT, "kT": kT, "vT": vT,
            "wq": np.ascontiguousarray(Wq[sl, :].T),
            "wk": np.ascontiguousarray(Wk[sl, :].T),
            "wv": np.ascontiguousarray(Wv[sl, :].T),
            "wo": np.ascontiguousarray(Wo[:, sl].T),
        }
        if n_pat:
            m["bias"] = bias_cat
        in_maps.append(m)

    res = run_bass_kernel_spmd(nc, in_maps, list(range(N_CORES)))
    global last_run
    last_run = res
    out_T = np.zeros((D, B * S), dtype=np.float32)
    for c in range(N_CORES):
        out_T += res.results[c]["pout"]
    return np.ascontiguousarray(out_T.T).reshape(B, S, D)


last_run = None


# revision 29
# speedup vs baseline: 1.0321x; 1.0278x over previous
"""Multi-head attention (B=2, S=2048, D=1024, H=16) on 8 TRN2 NeuronCores.

Sharding: tensor-parallel over heads. Core c computes heads {2c, 2c+1}:
  - q/k/v projections restricted to its 128 output dims (full q/k/v
    streamed per core, rank-128 weight slices),
  - causal flash attention for its 2 heads (both batches), scores kept in
    [k_token, q_token] layout so no transposes are needed,
  - partial output projection with its 128-column slice of Wo.
Host sums the 8 partial outputs.

All matmuls run as float32r (TRN2's full-rate fp32 mode; ~1.5e-4 rel err
vs float64 at K=1024 — measured identical to the PE's plain fp32 mode,
at 1 cycle/row instead of 4). Softmax skips the max-subtraction (scores
are O(+-2) for sane inputs) and gets denominators for free from a
ones-augmented V (M=65 matmul: rows 0-63 = sum(P*V), row 64 = sum(P)).
Causal masking zeroes the exp tiles on diagonal-crossing k-tiles with a
single GpSimd affine_select each; fully-masked tiles are skipped. The
mask handling is derived from the actual mask input on the host, so any
mask pattern (causal, full, block-sparse, non-affine) works.

The emission schedule is a single fused software pipeline per
(batch, 512-token q-block): scores->exp->PV beats with the AV matmul
lagging one beat behind the exp, and the next chunk's projection runs,
the previous block's normalize + output projection, and the V-transpose
interleaved into the beats so the PE instruction stream stays dense
(keeps the PE_HAM clock gate at full speed).
"""

import sys
import types

import numpy as np

# If tracing is requested (BASS_TRACE) but the image's antenv lacks the
# axon_hooks module, register a ctypes-based NTFF hook so the trace path
# in run_bass_kernel_spmd does not crash.
try:
    import antenv.axon_hooks  # noqa: F401
except ImportError:
    try:
        from trn_agent_boot.trn_boot import _ntff_profile_via_ctypes
        _m = types.ModuleType("antenv.axon_hooks")
        _hook = _ntff_profile_via_ctypes("/opt/axon/libaxon_pjrt.so")
        _m.get_axon_ntff_profile_hook = lambda: _hook
        _m.set_axon_ntff_profile_hook = lambda h: None
        sys.modules["antenv.axon_hooks"] = _m
    except Exception:
        pass

import concourse.mybir as mybir
from concourse import bacc
from concourse.tile import TileContext
from concourse.bass_utils import run_bass_kernel_spmd
from concourse.masks import make_identity

B, S, D, H, HD = 2, 2048, 1024, 16, 64
N_CORES = 8
DLOC = D // N_CORES          # 128 dims (2 heads) per core
SCALE = 1.0 / np.sqrt(HD)
TBLK = 512                   # q-token block (ST free dim)
JBLK = 128                   # k-token tile (ST partition dim)
NTB = S // TBLK              # 4 t-blocks per batch
NJT = S // JBLK              # 16 j-tiles per batch
ET = D // 128                # 8 contraction e-tiles
NEG = -1.0e30

F32 = mybir.dt.float32
F32R = mybir.dt.float32r

_compiled = {}


def _classify(mask2d):
    """Per (t-block, j-tile): None (fully valid), skipped (fully masked),
    ("affine", r) when the valid set is {(jj,tt): tt >= jj + r},
    else ("bias", idx) into a dedup'd list of [128 j, 512 t] bias tiles."""
    pats = {}
    pat_list = []
    table = []
    jj = np.arange(JBLK)[:, None]
    tt = np.arange(TBLK)[None, :]
    for tb in range(NTB):
        t0 = tb * TBLK
        row = []
        for j0 in range(0, S, JBLK):
            blk = mask2d[t0:t0 + TBLK, j0:j0 + JBLK]  # [t, j]
            if blk.all():
                row.append((j0, None))
                continue
            if not blk.any():
                continue
            valid = blk.T != 0  # [jj, tt]
            # affine candidate: r from the first row's first valid column
            first = np.argmax(valid, axis=1)  # per jj: first valid tt (if any)
            r = int(first[0])
            if np.array_equal(valid, tt >= jj + r):
                row.append((j0, ("affine", r)))
            else:
                bias = np.where(valid, np.float32(0.0),
                                np.float32(NEG)).astype(np.float32)
                key = bias.tobytes()
                if key not in pats:
                    pats[key] = len(pat_list)
                    pat_list.append(bias)
                row.append((j0, ("bias", pats[key])))
        table.append(tuple(row))
    return tuple(table), pat_list


def _build(table, n_pat):
    nc = bacc.Bacc(None, num_devices=N_CORES)
    qT_ext = nc.declare_dram_parameter("qT", [D, B * S], F32R, isOutput=False)
    kT_ext = nc.declare_dram_parameter("kT", [D, B * S], F32R, isOutput=False)
    vT_ext = nc.declare_dram_parameter("vT", [D, B * S], F32R, isOutput=False)
    wq_ext = nc.declare_dram_parameter("wq", [D, DLOC], F32R, isOutput=False)
    wk_ext = nc.declare_dram_parameter("wk", [D, DLOC], F32R, isOutput=False)
    wv_ext = nc.declare_dram_parameter("wv", [D, DLOC], F32R, isOutput=False)
    wo_ext = nc.declare_dram_parameter("wo", [DLOC, D], F32R, isOutput=False)
    if n_pat:
        bias_ext = nc.declare_dram_parameter("bias", [JBLK, n_pat * TBLK], F32,
                                             isOutput=False)
    pout_ext = nc.declare_dram_parameter("pout", [D, B * S], F32, isOutput=True)

    with TileContext(nc) as tc:
        with (
            tc.tile_pool(name="const", bufs=1) as const,
            tc.tile_pool(name="big", bufs=1) as big,
            tc.tile_pool(name="inp", bufs=2) as inp,
            tc.tile_pool(name="work", bufs=3) as work,
            tc.tile_pool(name="ps", bufs=1, space="PSUM") as ps,
        ):
            # ---------- constants ----------
            wq_sb = const.tile([128, ET * DLOC], F32R, tag="wq")
            wk_sb = const.tile([128, ET * DLOC], F32R, tag="wk")
            wv_sb = const.tile([128, ET * DLOC], F32R, tag="wv")
            for w_sb, w_ext in ((wq_sb, wq_ext), (wk_sb, wk_ext), (wv_sb, wv_ext)):
                nc.sync.dma_start(
                    out=w_sb[:].rearrange("p (a d) -> p a d", a=ET),
                    in_=w_ext.rearrange("(a p) d -> p a d", p=128))
            wo_sb = const.tile([DLOC, D], F32R, tag="wo")
            nc.sync.dma_start(out=wo_sb[:], in_=wo_ext[:, :])
            if n_pat:
                bias_sb = const.tile([JBLK, n_pat * TBLK], F32, tag="bias")
                nc.sync.dma_start(out=bias_sb[:], in_=bias_ext[:, :])
            ident = const.tile([128, 128], F32, tag="ident")
            make_identity(nc, ident[:])
            ones32 = const.tile([128, NJT], F32, tag="ones32")
            nc.vector.memset(ones32[:], 1.0)

            # ---------- persistent per-core activations ----------
            qhT_sb = big.tile([128, B * S], F32R, tag="qhT")
            khT_sb = big.tile([128, B * S], F32R, tag="khT")
            vaug_sb = big.tile([128, B * 2 * NJT * 65], F32R, tag="vaug")
            attn2_sb = big.tile([128, B * S], F32R, tag="attn2")

            def vaug_sl(b, h, j0, c0, c1):
                base = ((b * 2 + h) * NJT + j0 // JBLK) * 65
                return vaug_sb[:, base + c0:base + c1]

            for b in range(B):
                for h in range(2):
                    base = (b * 2 + h) * NJT * 65 + 64
                    dst = vaug_sb[:, base:base + 65 * (NJT - 1) + 1:65]
                    nc.vector.tensor_copy(dst, ones32[:])

            av_tiles = {}

            # ---------- building blocks ----------
            def chunk_dmas_one(b, tch, nm, ext):
                col0 = b * S + tch * TBLK
                grp = []
                for e in range(ET):
                    t_in = inp.tile([128, TBLK], F32R, tag="xin", bufs=24,
                                    name=f"in_{nm}_{b}_{tch}_{e}")
                    nc.sync.dma_start(
                        out=t_in[:],
                        in_=ext[e * 128:(e + 1) * 128, col0:col0 + TBLK])
                    grp.append(t_in)
                return {nm: grp}

            def chunk_dmas(b, tch):
                """Queue the 24 input-tile DMAs for chunk (b, tch)."""
                tiles = {}
                for nm, ext in (("q", qT_ext), ("k", kT_ext), ("v", vT_ext)):
                    tiles.update(chunk_dmas_one(b, tch, nm, ext))
                return tiles

            def proj_half(b, tch, nm, tiles, half, state):
                """Half of a projection accumulation run (4 matmuls); the
                second half runs the epilogue."""
                col0 = b * S + tch * TBLK
                w_sb = {"q": wq_sb, "k": wk_sb, "v": wv_sb}[nm]
                if half == 0:
                    pp = ps.tile([128, TBLK], F32, tag="pc", bufs=2,
                                 name=f"pp_{nm}_{b}_{tch}")
                    state[nm] = pp
                else:
                    pp = state[nm]
                for e in range(half * 4, half * 4 + 4):
                    nc.tensor.matmul(pp[:], lhsT=w_sb[:, e * DLOC:(e + 1) * DLOC],
                                     rhs=tiles[e][:],
                                     start=(e == 0), stop=(e == ET - 1))
                if half == 0:
                    return
                if nm == "q":
                    nc.vector.tensor_copy(qhT_sb[:, col0:col0 + TBLK], pp[:])
                elif nm == "k":
                    nc.vector.tensor_copy(khT_sb[:, col0:col0 + TBLK], pp[:])
                else:
                    vt = work.tile([128, TBLK], F32, tag="vt", bufs=3)
                    nc.vector.tensor_copy(vt[:], pp[:])
                    for kb in range(TBLK // 128):
                        tr = ps.tile([128, 128], F32, tag="pc", bufs=2,
                                     name=f"tr_{b}_{tch}_{kb}")
                        nc.tensor.transpose(tr[:], vt[:, kb * 128:(kb + 1) * 128],
                                            ident[:])
                        j0 = tch * TBLK + kb * 128
                        nc.vector.tensor_copy(vaug_sl(b, 0, j0, 0, 64), tr[:, 0:64])
                        nc.vector.tensor_copy(vaug_sl(b, 1, j0, 0, 64), tr[:, 64:128])

            def fin_normalize(b, tb):
                """Divide by the ones-row denominator into attn2_sb; head B
                lands on partitions 64-127 via a small SBUF shift DMA."""
                cc = b * S + tb * TBLK
                av = av_tiles[(b, tb)]
                for h in range(2):
                    den = work.tile([65, TBLK], F32, tag="den", bufs=2)
                    nc.vector.tensor_copy(den[64:65, :], av[h][64:65, :])
                    sh = work.tile([1, TBLK], F32, tag="sh", bufs=2)
                    nc.sync.dma_start(out=sh[:], in_=den[64:65, :])
                    bc = work.tile([64, TBLK], F32, tag="bc", bufs=2)
                    nc.gpsimd.partition_broadcast(bc[:], sh[:])
                    bcr = work.tile([64, TBLK], F32, tag="bcr", bufs=2)
                    nc.vector.reciprocal_approx_fast(bcr[:], bc[:])
                    if h == 0:
                        nc.vector.tensor_mul(attn2_sb[0:64, cc:cc + TBLK],
                                             av[h][0:64, :], bcr[:])
                    else:
                        tmpb = work.tile([64, TBLK], F32R, tag="tmpb", bufs=2)
                        nc.vector.tensor_mul(tmpb[:], av[h][0:64, :], bcr[:])
                        nc.sync.dma_start(out=attn2_sb[64:128, cc:cc + TBLK],
                                          in_=tmpb[:])

            def po_group(b, tb, e):
                """Partial out-projection for e-tile e of block (b, tb)."""
                cc = b * S + tb * TBLK
                po = ps.tile([128, TBLK], F32, tag="pc", bufs=2,
                             name=f"po_{b}_{tb}_{e}")
                nc.tensor.matmul(po[:], lhsT=wo_sb[:, e * 128:(e + 1) * 128],
                                 rhs=attn2_sb[:, cc:cc + TBLK],
                                 start=True, stop=True)
                pos = work.tile([128, TBLK], F32, tag="pos", bufs=4)
                nc.vector.tensor_copy(pos[:], po[:])
                nc.gpsimd.dma_start(out=pout_ext[e * 128:(e + 1) * 128, cc:cc + TBLK],
                                    in_=pos[:])

            # ---------- fused pipeline ----------
            def block(b, tb, prev, nxt):
                """Attention for (b, tb) with prev's finish and nxt's
                projection interleaved into the beats."""
                cc = b * S + tb * TBLK
                row = table[tb]
                n_j = len(row)
                if prev is not None:
                    fin_normalize(*prev)
                fillers = []
                if nxt is not None:
                    tiles = chunk_dmas(*nxt)
                    pstate = {}
                    for nm in ("q", "k", "v"):
                        for half in range(2):
                            fillers.append(
                                lambda nm=nm, half=half:
                                proj_half(nxt[0], nxt[1], nm, tiles[nm], half, pstate))
                if prev is not None:
                    for e in range(ET):
                        fillers.append(lambda pb=prev[0], pt=prev[1], pe=e:
                                       po_group(pb, pt, pe))
                av = [ps.tile([65, TBLK], F32, tag="p1a", name=f"avA_{b}_{tb}"),
                      ps.tile([65, TBLK], F32, tag="p1b", name=f"avB_{b}_{tb}")]
                av_tiles[(b, tb)] = av

                pend = []
                nfill = len(fillers)
                fi = 0
                for ji, (j0, m) in enumerate(row):
                    jc = b * S + j0
                    # leading fully-masked columns of an affine tile: skip
                    r = m[1] if (m is not None and m[0] == "affine") else 0
                    r = max(0, min(r, TBLK - 128))  # keep at least 128 cols
                    stA = ps.tile([128, TBLK], F32, tag="p2a", bufs=2)
                    stB = ps.tile([128, TBLK], F32, tag="p2b", bufs=2)
                    nc.tensor.matmul(stA[:, r:], lhsT=khT_sb[0:64, jc:jc + JBLK],
                                     rhs=qhT_sb[0:64, cc + r:cc + TBLK],
                                     start=True, stop=True)
                    nc.tensor.matmul(stB[:, r:], lhsT=khT_sb[64:128, jc:jc + JBLK],
                                     rhs=qhT_sb[64:128, cc + r:cc + TBLK],
                                     start=True, stop=True)
                    exps = []
                    for h, sth in enumerate((stA, stB)):
                        if m is not None and m[0] == "bias":
                            nc.vector.tensor_add(
                                sth[:], sth[:],
                                bias_sb[:, m[1] * TBLK:(m[1] + 1) * TBLK])
                        ex = work.tile([128, TBLK], F32R, tag=f"exp{h}", bufs=4)
                        nc.scalar.activation(ex[:, r:], sth[:, r:],
                                             mybir.ActivationFunctionType.Exp,
                                             scale=float(SCALE))
                        if m is not None and m[0] == "affine":
                            # keep element iff (tt-r) >= jj + (m[1]-r), else 0
                            nc.gpsimd.affine_select(
                                out=ex[:, r:], in_=ex[:, r:],
                                pattern=[[1, TBLK - r]],
                                compare_op=mybir.AluOpType.is_ge,
                                fill=0.0, base=-(m[1] - r), channel_multiplier=-1)
                        exps.append(ex)
                    pend.append((exps, j0, ji, r))
                    if len(pend) > 2:
                        pexps, pj0, pji, pr = pend.pop(0)
                        for h in range(2):
                            nc.tensor.matmul(av[h][:, pr:],
                                             lhsT=vaug_sl(b, h, pj0, 0, 65),
                                             rhs=pexps[h][:, pr:],
                                             start=(pji == 0), stop=False)
                    # interleave filler work to keep the PE stream dense
                    want = (ji + 1) * nfill // (n_j + 2)
                    while fi < want:
                        fillers[fi]()
                        fi += 1
                while pend:
                    pexps, pj0, pji, pr = pend.pop(0)
                    for h in range(2):
                        nc.tensor.matmul(av[h][:, pr:],
                                         lhsT=vaug_sl(b, h, pj0, 0, 65),
                                         rhs=pexps[h][:, pr:],
                                         start=(pji == 0), stop=(pji == n_j - 1))
                while fi < nfill:
                    fillers[fi]()
                    fi += 1

            # prologue: project chunk (0, 0)
            tiles0 = chunk_dmas(0, 0)
            pstate0 = {}
            for nm in ("q", "k", "v"):
                for half in range(2):
                    proj_half(0, 0, nm, tiles0[nm], half, pstate0)
            # fused blocks
            blocks = [(b, tb) for b in range(B) for tb in range(NTB)]
            for i, (b, tb) in enumerate(blocks):
                prev = blocks[i - 1] if i > 0 else None
                nxt = blocks[i + 1] if i + 1 < len(blocks) else None
                block(b, tb, prev, nxt)
            # epilogue: finish the last block
            last = blocks[-1]
            fin_normalize(*last)
            for e in range(ET):
                po_group(last[0], last[1], e)

    nc.finalize()
    return nc


def kernel(q, k, v, mask, Wq, Wk, Wv, Wo):
    q = np.asarray(q, dtype=np.float32)
    k = np.asarray(k, dtype=np.float32)
    v = np.asarray(v, dtype=np.float32)
    mask2d = np.asarray(mask).reshape(S, S)
    Wq = np.asarray(Wq, dtype=np.float32)
    Wk = np.asarray(Wk, dtype=np.float32)
    Wv = np.asarray(Wv, dtype=np.float32)
    Wo = np.asarray(Wo, dtype=np.float32)

    table, pat_list = _classify(mask2d)
    n_pat = len(pat_list)
    key = (table, n_pat)
    if key not in _compiled:
        _compiled[key] = _build(table, n_pat)
    nc = _compiled[key]

    qT = np.ascontiguousarray(q.reshape(B * S, D).T)
    kT = np.ascontiguousarray(k.reshape(B * S, D).T)
    vT = np.ascontiguousarray(v.reshape(B * S, D).T)
    if n_pat:
        bias_cat = np.concatenate(pat_list, axis=1)

    in_maps = []
    for c in range(N_CORES):
        sl = slice(c * DLOC, (c + 1) * DLOC)
        m = {
            "qT": qT, "kT": kT, "vT": vT,
            "wq": np.ascontiguousarray(Wq[sl, :].T),
            "wk": np.ascontiguousarray(Wk[sl, :].T),
            "wv": np.ascontiguousarray(Wv[sl, :].T),
            "wo": np.ascontiguousarray(Wo[:, sl].T),
        }
        if n_pat:
            m["bias"] = bias_cat
        in_maps.append(m)

    res = run_bass_kernel_spmd(nc, in_maps, list(range(N_CORES)))
    global last_run
    last_run = res
    out_T = np.zeros((D, B * S), dtype=np.float32)
    for c in range(N_CORES):
        out_T += res.results[c]["pout"]
    return np.ascontiguousarray(out_T.T).reshape(B, S, D)


last_run = None
